# revision 70
# baseline (speedup 1.0000x reference)
"""ColorMLP Trainium2 kernel (v3 — DMA-transpose + 4-way engine balance).

Reference computation (per pixel, 8 input channels):
    h1 = relu(x @ w0 + b0)         # 8 -> 16
    h2 = relu(h1 @ w1 + b1)        # 16 -> 16
    y  = sigmoid(h2 @ w2 + b2)     # 16 -> 3
    out = mask * ((1-res)*rgb + res*y)   rgb = x[..,:3], res = x[..,3]

Strategy (pure data parallel over 8 cores, 1,048,576 px each):
  16 batches x 65536 px; partition p owns pixels [p*8192, (p+1)*8192);
  batch b covers within-partition offsets [b*512, (b+1)*512).

  - x: per-batch SWDGE cast-DMA f32->bf16 pixel-major [128, 4096].
  - transpose to feature-major via XBAR DMA transpose (SBUF->SBUF, no PE
    rows, no PSUM evac): t[j, c, g] = x_bf[g, 128c+j], j = 8*slot + feat.
  - L0/L1 matmuls f32 PSUM in [128,1024] tiles; relu evacs split DVE/ACT
    per the tunable EVAC_ENGINES map (f32 PSUM src => no DVE 2x; ACT is
    0.833 ns/elem, DVE 1.04).
  - L2 fused with output transpose into z halves [128,768] f32; ACT
    sigmoid -> y_sb bf16.
  - blend: bco/aco/u/v on Pool (dtype-blind cost => f32 products for
    accuracy), o = u + v on DVE f32.
  - store o f32 via HWDGE.
"""

import os
import sys

import numpy as np

sys.path.insert(0, "/opt/trn_rl_repo")

import ml_dtypes

import concourse.bacc as bacc
import concourse.bass as bass
import concourse.mybir as mybir
import concourse.tile as tile
from concourse.bass_utils import run_bass_kernel_spmd

F32 = mybir.dt.float32
BF16 = mybir.dt.bfloat16
U8 = mybir.dt.uint8

N_CORES = 8
B, H, W = 8, 1024, 1024
IN_DIM, HID, OUT_DIM = 8, 16, 3
NPX = B * H * W                  # 8388608
NPC = NPX // N_CORES             # 1048576 per core
PPPC = NPC // 128                # 8192 pixels per partition per core

BATCH_PX = 65536                 # pixels per batch
PPP = BATCH_PX // 128            # 512 px per partition per batch
NCHUNK = PPP // 16               # 32 transpose chunks per batch

# relu-evac engine split: 16 evac tiles of [128,1024] per batch, indexed
# tix*2+s (L0) and 8+tix*2+s (L1).  True -> ACT, False -> DVE,
# "split" -> 512 cols on each engine.  Balance point is ~8.4 tiles on
# ACT (ACT 0.833/elem + sigmoids vs DVE 1.04/elem + v).
EVAC_ON_ACT = [
    True, False, True, False,
    True, False, True, True,
    True, False, True, False,
    True, False, False, True,
]


def _bd(w, reps):
    """Block-diagonal of `w` repeated `reps` times: [reps*K, reps*M]."""
    k, m = w.shape
    out = np.zeros((reps * k, reps * m), np.float32)
    for g in range(reps):
        out[g * k:(g + 1) * k, g * m:(g + 1) * m] = w
    return out


def _prep_weights(w0, b0, w1, b1, w2, b2):
    """Host-side constant prep. Returns dict of named numpy arrays."""
    bf = ml_dtypes.bfloat16
    bd0 = _bd(w0, 8)  # [64, 128]
    w0t = np.concatenate([bd0, bd0], axis=0)  # [128, 128]
    w1bd = _bd(w1, 8)  # [128, 128]
    g2 = _bd(w2, 8)    # [128, 24]
    b0col = np.tile(b0, 8).astype(np.float32).reshape(128, 1)
    b1col = np.tile(b1, 8).astype(np.float32).reshape(128, 1)
    b2row = np.tile(b2, PPP).astype(np.float32).reshape(1, PPP * 3)
    return {
        "W0T": w0t.astype(bf),
        "W1BD": w1bd.astype(bf),
        "G2": g2.astype(bf),
        "B0COL": b0col,
        "B1COL": b1col,
        "B2ROW": b2row,
        "b01_nonzero": bool(np.any(b0 != 0.0) or np.any(b1 != 0.0)),
        "b2_nonzero": bool(np.any(b2 != 0.0)),
    }


def build_program(npc, b01_nonzero, b2_nonzero, repeat=1):
    """Build the SPMD Bass program for one core processing `npc` pixels."""
    nc = bacc.Bacc("TRN2", target_bir_lowering=False, debug=False,
                   num_devices=N_CORES)
    n_batch = npc // BATCH_PX
    pppc = npc // 128

    x_d = nc.dram_tensor("x", [npc, IN_DIM], F32, kind="ExternalInput")
    m_d = nc.dram_tensor("mask", [npc], U8, kind="ExternalInput")
    w0t_d = nc.dram_tensor("W0T", [128, 128], BF16, kind="ExternalInput")
    w1bd_d = nc.dram_tensor("W1BD", [128, 128], BF16, kind="ExternalInput")
    g2_d = nc.dram_tensor("G2", [128, 24], BF16, kind="ExternalInput")
    b0_d = nc.dram_tensor("B0COL", [128, 1], F32, kind="ExternalInput")
    b1_d = nc.dram_tensor("B1COL", [128, 1], F32, kind="ExternalInput")
    b2_d = nc.dram_tensor("B2ROW", [1, PPP * 3], F32, kind="ExternalInput")
    out_d = nc.dram_tensor("out", [npc, OUT_DIM], F32, kind="ExternalOutput")

    # DRAM views — partition-contiguous pixel map.
    x_v = x_d[:].rearrange("(p b n) f -> b p (n f)", p=128, b=n_batch)
    m_v = m_d[:].rearrange("(p n) -> p n", p=128)
    o_v = out_d[:].rearrange("(p b n) c -> b p (n c)", p=128, b=n_batch)

    A = mybir.AluOpType

    with tile.TileContext(nc) as tc:
        with (
            tc.tile_pool(name="consts", bufs=1) as cpool,
            tc.tile_pool(name="xin", bufs=3) as xpool,
            tc.tile_pool(name="tsb", bufs=3) as tpool,
            tc.tile_pool(name="hsb", bufs=2) as hpool,
            tc.tile_pool(name="h2sb", bufs=2) as h2pool,
            tc.tile_pool(name="ysb", bufs=2) as ypool,
            tc.tile_pool(name="co", bufs=2) as copool,
            tc.tile_pool(name="uv", bufs=2) as uvpool,
            tc.tile_pool(name="osb", bufs=2) as opool,
            tc.tile_pool(name="mmps", bufs=4, space="PSUM") as mmps_pool,
        ):
            # ---- constants + whole-core mask (u8 -> bf16 cast DMA) ----
            w0t = cpool.tile([128, 128], BF16, tag="w0t")
            w1bd = cpool.tile([128, 128], BF16, tag="w1bd")
            g2 = cpool.tile([128, 24], BF16, tag="g2")
            mask_sb = cpool.tile([128, pppc], BF16, tag="mask")
            nc.sync.dma_start(w0t[:], w0t_d[:])
            nc.sync.dma_start(w1bd[:], w1bd_d[:])
            nc.sync.dma_start(g2[:], g2_d[:])
            if b01_nonzero:
                b0c = cpool.tile([128, 1], F32, tag="b0c")
                b1c = cpool.tile([128, 1], F32, tag="b1c")
                nc.sync.dma_start(b0c[:], b0_d[:])
                nc.sync.dma_start(b1c[:], b1_d[:])
            if b2_nonzero:
                b2r = cpool.tile([1, PPP * 3], F32, tag="b2r")
                nc.sync.dma_start(b2r[:], b2_d[:])

            def relu_evac(dst, src, bias_tile, on_act):
                if on_act == "split":
                    n = src.shape[-1] // 2
                    relu_evac(dst[:, :n], src[:, :n], bias_tile, True)
                    relu_evac(dst[:, n:], src[:, n:], bias_tile, False)
                    return
                if on_act:
                    bias = bias_tile[:] if bias_tile is not None else 0.0
                    nc.scalar.activation(
                        dst, src, mybir.ActivationFunctionType.Relu,
                        bias=bias)
                else:
                    s1 = bias_tile[:] if bias_tile is not None else 0.0
                    nc.vector.tensor_scalar(
                        out=dst, in0=src, scalar1=s1, scalar2=0.0,
                        op0=A.add, op1=A.max)

            # ---- batch schedule: uniform within-partition pixel ranges
            # (ramped batch sizes were tried and hurt the steady state) ----
            sizes = [PPP] * (pppc // PPP)
            assert sum(sizes) == pppc
            spans = []
            off = 0
            for sz in sizes:
                spans.append((off, sz))
                off += sz
            batches = [sp for _ in range(repeat) for sp in spans]

            x_pn = x_d[:].rearrange("(p n) f -> p (n f)", p=128)
            o_pn = out_d[:].rearrange("(p n) c -> p (n c)", p=128)

            x_tiles = {}

            def load_x(i):
                # SWDGE cast f32->bf16, pixel-major
                off, ppb = batches[i]
                x_bf = xpool.tile([128, PPP * IN_DIM], BF16, tag="x")
                nc.gpsimd.dma_start(
                    x_bf[:, :ppb * IN_DIM],
                    x_pn[:, off * IN_DIM:(off + ppb) * IN_DIM])
                x_tiles[i] = x_bf

            def emit_tail(prev, nsplit=1):
                assert prev[0][1] % nsplit == 0
                """v = bco*y (DVE), o = u+v (Pool), store (SP) for a
                FINISHED batch — software-pipelined into the next batch's
                schedule so the sigmoid->v->o chain is off the critical
                inter-batch cycle.  nsplit>1 pipelines the v->o->store
                chain in column chunks (used for the final drain)."""
                (off, ppb), y_p, u_p, bco_p = prev
                v_sb = uvpool.tile([128, PPP * 3], F32, tag="v")
                o_sb = opool.tile([128, PPP * 3], F32, tag="o")
                pc = ppb // nsplit
                for k in range(nsplit):
                    p0, p1 = k * pc, (k + 1) * pc
                    nc.vector.tensor_tensor(
                        out=v_sb[:, p0 * 3:p1 * 3]
                            .rearrange("p (n c) -> p n c", c=3),
                        in0=y_p[:, p0 * 3:p1 * 3]
                            .rearrange("p (n c) -> p n c", c=3),
                        in1=bco_p[:, p0:p1].unsqueeze(2).broadcast_to(
                            [128, pc, 3]),
                        op=A.mult)
                    nc.gpsimd.tensor_tensor(
                        out=o_sb[:, p0 * 3:p1 * 3], in0=u_p[:, p0 * 3:p1 * 3],
                        in1=v_sb[:, p0 * 3:p1 * 3], op=A.add)
                    nc.sync.dma_start(
                        o_pn[:, (off + p0) * 3:(off + p1) * 3],
                        o_sb[:, p0 * 3:p1 * 3])

            load_x(0)
            # mask (u8 -> bf16 cast) split: a small head chunk so batch 0
            # isn't stuck behind the whole-core transfer, then the rest.
            mh = spans[0][1] + spans[1][1] if len(spans) > 1 else pppc
            nc.gpsimd.dma_start(mask_sb[:, :mh], m_v[:, :mh])
            load_x(1)
            if mh < pppc:
                nc.gpsimd.dma_start(mask_sb[:, mh:], m_v[:, mh:])
            prev = None  # ((off, ppb), y_sb, u_sb, bco) of previous batch

            for i, (off, ppb) in enumerate(batches):
                x_bf = x_tiles.pop(i)
                ncols = ppb * IN_DIM           # x/t cols this batch
                # evac tile count per s-half per layer; tile width is
                # always 1024 cols since ppb is a multiple of 128
                assert ppb % 128 == 0
                ntix = ppb // 128

                # ---- feature-major transpose via XBAR DMA (SP queue) ----
                # t[j, c, g] = x_bf[g, 128c + j]; j = 8*slot + feat
                t_sb = tpool.tile([128, PPP * IN_DIM], BF16, tag="t")
                nc.sync.dma_start_transpose(
                    t_sb[:, :ncols].rearrange("j (c g) -> j c g", g=128),
                    x_bf[:, :ncols].rearrange("p (c j) -> p c j", j=128),
                )

                # ---- previous batch's blend tail + store ----
                if prev is not None:
                    emit_tail(prev)

                # ---- prefetch next batch's x first in Pool's stream, so
                # its SWDGE desc-gen isn't queued behind this batch's TTs ----
                if i + 1 < len(batches) and i + 1 not in x_tiles:
                    load_x(i + 1)

                # ---- blend coefficients early, on Pool ----
                x3 = x_bf[:, :ncols].rearrange("p (n f) -> p n f", f=IN_DIM)
                rgb = x3[:, :, 0:3]
                res = x3[:, :, 3]
                mk = mask_sb[:, off:off + ppb]
                bco = copool.tile([128, PPP], BF16, tag="bc")
                aco = copool.tile([128, PPP], BF16, tag="ac")
                u_sb = uvpool.tile([128, PPP * 3], F32, tag="u")
                nc.gpsimd.tensor_tensor(out=bco[:, :ppb], in0=res, in1=mk,
                                        op=A.mult)
                nc.gpsimd.tensor_tensor(out=aco[:, :ppb], in0=mk,
                                        in1=bco[:, :ppb], op=A.subtract)
                nc.gpsimd.tensor_tensor(
                    out=u_sb[:, :ppb * 3].rearrange("p (n c) -> p n c", c=3),
                    in0=rgb,
                    in1=aco[:, :ppb].unsqueeze(2).broadcast_to([128, ppb, 3]),
                    op=A.mult)

                b0t = b0c if b01_nonzero else None
                b1t = b1c if b01_nonzero else None

                # ---- L0/L1 with interleaved s-halves ----
                h_sb = {}
                h2_sb = {}
                for s in range(2):
                    h_cur = hpool.tile([128, 4096], BF16, tag=f"hs{s}")
                    h_sb[s] = h_cur
                    h2_cur = h2pool.tile([128, 4096], BF16, tag=f"h2s{s}")
                    h2_sb[s] = h2_cur

                tw = 1024                      # evac tile width (cols)
                for tix in range(ntix):
                    for s in range(2):
                        h_ps = mmps_pool.tile([128, 1024], F32, tag="mm")
                        for q in range(2):
                            lo = tix * tw + q * (tw // 2)
                            nc.tensor.matmul(
                                h_ps[:, q * (tw // 2):(q + 1) * (tw // 2)],
                                lhsT=w0t[s * 64:(s + 1) * 64, :],
                                rhs=t_sb[s * 64:(s + 1) * 64, lo:lo + tw // 2],
                            )
                        relu_evac(h_sb[s][:, tix * tw:(tix + 1) * tw],
                                  h_ps[:, :tw], b0t,
                                  EVAC_ON_ACT[(tix * 2 + s) % 8])

                y_sb = ypool.tile([128, PPP * 3], BF16, tag="y")
                cdone = 0  # chunks L2-emitted so far
                for tix in range(ntix):
                    for s in range(2):
                        h2_ps = mmps_pool.tile([128, 1024], F32, tag="mm")
                        for q in range(2):
                            lo = tix * tw + q * (tw // 2)
                            nc.tensor.matmul(
                                h2_ps[:, q * (tw // 2):(q + 1) * (tw // 2)],
                                lhsT=w1bd[:],
                                rhs=h_sb[s][:, lo:lo + tw // 2],
                            )
                        relu_evac(h2_sb[s][:, tix * tw:(tix + 1) * tw],
                                  h2_ps[:, :tw], b1t,
                                  EVAC_ON_ACT[8 + (tix * 2 + s) % 8])

                    # ---- L2 + sigmoid for the chunk range now complete
                    # (z tile [128,768] = up to 16 chunks) ----
                    if tix % 2 == 1 or tix == ntix - 1:
                        cend = (tix + 1) * tw // 128
                        nch = cend - cdone
                        z_ps = mmps_pool.tile([128, 1024], F32, tag="mm")
                        for s in range(2):
                            for cl in range(nch):
                                cc = cdone + cl
                                col = cl * 48 + s * 24
                                nc.tensor.matmul(
                                    z_ps[:, col:col + 24],
                                    lhsT=h2_sb[s][:, cc * 128:(cc + 1) * 128],
                                    rhs=g2[:],
                                )
                        if b2_nonzero:
                            nc.vector.tensor_tensor(
                                out=z_ps[:, :nch * 48],
                                in0=z_ps[:, :nch * 48],
                                in1=b2r[:, cdone * 48:cend * 48]
                                    .partition_broadcast(128),
                                op=A.add)
                        nc.scalar.activation(
                            y_sb[:, cdone * 48:cend * 48], z_ps[:, :nch * 48],
                            mybir.ActivationFunctionType.Sigmoid,
                        )
                        cdone = cend

                prev = ((off, ppb), y_sb, u_sb, bco)

            # final batch's tail, split to pipeline the drain
            emit_tail(prev, nsplit=4)

    nc.finalize()
    return nc


_PROGRAM_CACHE = {}


def _get_program(npc, b01_nonzero, b2_nonzero, repeat=1):
    key = (npc, b01_nonzero, b2_nonzero, repeat)
    if key not in _PROGRAM_CACHE:
        _PROGRAM_CACHE[key] = build_program(npc, b01_nonzero, b2_nonzero,
                                            repeat=repeat)
    return _PROGRAM_CACHE[key]


def _shard_inputs(x, mask):
    x_flat = np.ascontiguousarray(np.asarray(x, np.float32).reshape(NPX, IN_DIM))
    m_flat = np.ascontiguousarray(np.asarray(mask).astype(np.uint8).reshape(NPX))
    return x_flat, m_flat


def kernel(x, mask, w0, b0, w1, b1, w2, b2):
    consts = _prep_weights(
        np.asarray(w0, np.float32), np.asarray(b0, np.float32),
        np.asarray(w1, np.float32), np.asarray(b1, np.float32),
        np.asarray(w2, np.float32), np.asarray(b2, np.float32))
    x_flat, m_flat = _shard_inputs(x, mask)

    nc = _get_program(NPC, consts["b01_nonzero"], consts["b2_nonzero"])
    const_map = {k: np.asarray(v) for k, v in consts.items()
                 if k not in ("b2_nonzero", "b01_nonzero")}
    in_maps = []
    for k in range(N_CORES):
        lo, hi = k * NPC, (k + 1) * NPC
        in_maps.append({
            "x": x_flat[lo:hi],
            "mask": m_flat[lo:hi],
            **const_map,
        })
    res = run_bass_kernel_spmd(nc, in_maps, core_ids=list(range(N_CORES)))
    out = np.concatenate([res.results[k]["out"] for k in range(N_CORES)], axis=0)
    return out.reshape(B, H, W, OUT_DIM)


if __name__ == "__main__":
    rng = np.random.default_rng(0)
    x = rng.random((B, H, W, IN_DIM), np.float32)
    mask = rng.integers(0, 2, (B, H, W)).astype(bool)
    w0 = rng.standard_normal((IN_DIM, HID)).astype(np.float32) * 0.5
    b0 = np.zeros(HID, np.float32)
    w1 = rng.standard_normal((HID, HID)).astype(np.float32) * 0.3
    b1 = np.zeros(HID, np.float32)
    w2 = rng.standard_normal((HID, OUT_DIM)).astype(np.float32) * 0.3
    b2 = np.zeros(OUT_DIM, np.float32)
    out = kernel(x=x, mask=mask, w0=w0, b0=b0, w1=w1, b1=b1, w2=w2, b2=b2)
    print("out", out.shape, out.dtype, out[0, 0, :2])


# revision 82
# speedup vs baseline: 1.0456x; 1.0456x over previous
"""ColorMLP Trainium2 kernel (v4 — DMA-transpose, unified PSUM ring,
software-pipelined tail).

Reference computation (per pixel, 8 input channels):
    h1 = relu(x @ w0 + b0)         # 8 -> 16
    h2 = relu(h1 @ w1 + b1)        # 16 -> 16
    y  = sigmoid(h2 @ w2 + b2)     # 16 -> 3
    out = mask * ((1-res)*rgb + res*y)   rgb = x[..,:3], res = x[..,3]

Strategy (pure data parallel over 8 cores, 1,048,576 px each):
  16 batches x 65536 px; partition p owns pixels [p*8192, (p+1)*8192);
  batch b covers within-partition offsets [b*512, (b+1)*512).

  - x: per-batch SWDGE cast-DMA f32->bf16 pixel-major [128, 4096],
    prefetched one batch ahead at the head of Pool's stream.
  - transpose to feature-major via XBAR DMA transpose (SBUF->SBUF, no PE
    rows, no PSUM evac): t[j, c, g] = x_bf[g, 128c+j], j = 8*slot+feat.
    Fill-phase batches 0-1 instead transpose on PE (through the PSUM
    ring) so compute starts the moment their x lands.
  - ONE unified PSUM ring: [128,1024] f32 x 4 bufs = all 8 banks,
    shared by L0, L1 and the L2 z tiles — ring depth 4 hides the
    PE->evac->PE round-trip latency.
  - relu evacs (f32 PSUM -> bf16 SBUF) split 9 ACT / 7 DVE per
    EVAC_ON_ACT (ACT 0.833 ns/elem + 2 sigmoids, DVE 1.04 + v); the
    last L1 tile sits on ACT so its sigmoid follows in-order.
  - L2 fused with the output transpose into z halves [128,768]; ACT
    sigmoid -> y_sb bf16.
  - blend tail (v = bco*y on DVE, o = u + v on Pool, store on SP) is
    software-pipelined one batch late so the sigmoid->v->o chain stays
    off the inter-batch critical cycle; bco/aco/u run early on Pool.
    u/v/o in f32 (Pool cost is dtype-blind) for accuracy: rel err
    0.0152 vs 0.0179 with bf16 products.
  - store o f32 via HWDGE; final batch's tail store split in quarters.
"""

import os
import sys

import numpy as np

sys.path.insert(0, "/opt/trn_rl_repo")

import ml_dtypes

import concourse.bacc as bacc
import concourse.bass as bass
import concourse.mybir as mybir
import concourse.tile as tile
from concourse.bass_utils import run_bass_kernel_spmd

F32 = mybir.dt.float32
BF16 = mybir.dt.bfloat16
U8 = mybir.dt.uint8

N_CORES = 8
B, H, W = 8, 1024, 1024
IN_DIM, HID, OUT_DIM = 8, 16, 3
NPX = B * H * W                  # 8388608
NPC = NPX // N_CORES             # 1048576 per core
PPPC = NPC // 128                # 8192 pixels per partition per core

BATCH_PX = 65536                 # pixels per batch
PPP = BATCH_PX // 128            # 512 px per partition per batch
NCHUNK = PPP // 16               # 32 transpose chunks per batch

# relu-evac engine split: 16 evac tiles of [128,1024] per batch, indexed
# tix*2+s (L0) and 8+tix*2+s (L1).  True -> ACT, False -> DVE,
# "split" -> 512 cols on each engine.  Balance point is ~8.4 tiles on
# ACT (ACT 0.833/elem + sigmoids vs DVE 1.04/elem + v).
EVAC_ON_ACT = [
    True, False, True, False,
    True, False, True, True,
    True, False, True, False,
    True, False, False, True,
]


def _bd(w, reps):
    """Block-diagonal of `w` repeated `reps` times: [reps*K, reps*M]."""
    k, m = w.shape
    out = np.zeros((reps * k, reps * m), np.float32)
    for g in range(reps):
        out[g * k:(g + 1) * k, g * m:(g + 1) * m] = w
    return out


def _prep_weights(w0, b0, w1, b1, w2, b2):
    """Host-side constant prep. Returns dict of named numpy arrays."""
    bf = ml_dtypes.bfloat16
    bd0 = _bd(w0, 8)  # [64, 128]
    w0t = np.concatenate([bd0, bd0], axis=0)  # [128, 128]
    w1bd = _bd(w1, 8)  # [128, 128]
    g2 = _bd(w2, 8)    # [128, 24]
    b0col = np.tile(b0, 8).astype(np.float32).reshape(128, 1)
    b1col = np.tile(b1, 8).astype(np.float32).reshape(128, 1)
    b2row = np.tile(b2, PPP).astype(np.float32).reshape(1, PPP * 3)
    return {
        "W0T": w0t.astype(bf),
        "W1BD": w1bd.astype(bf),
        "G2": g2.astype(bf),
        "IDENT": np.eye(128, dtype=np.float32).astype(bf),
        "B0COL": b0col,
        "B1COL": b1col,
        "B2ROW": b2row,
        "b01_nonzero": bool(np.any(b0 != 0.0) or np.any(b1 != 0.0)),
        "b2_nonzero": bool(np.any(b2 != 0.0)),
    }


def build_program(npc, b01_nonzero, b2_nonzero, repeat=1):
    """Build the SPMD Bass program for one core processing `npc` pixels."""
    nc = bacc.Bacc("TRN2", target_bir_lowering=False, debug=False,
                   num_devices=N_CORES)
    n_batch = npc // BATCH_PX
    pppc = npc // 128

    x_d = nc.dram_tensor("x", [npc, IN_DIM], F32, kind="ExternalInput")
    m_d = nc.dram_tensor("mask", [npc], U8, kind="ExternalInput")
    w0t_d = nc.dram_tensor("W0T", [128, 128], BF16, kind="ExternalInput")
    w1bd_d = nc.dram_tensor("W1BD", [128, 128], BF16, kind="ExternalInput")
    g2_d = nc.dram_tensor("G2", [128, 24], BF16, kind="ExternalInput")
    id_d = nc.dram_tensor("IDENT", [128, 128], BF16, kind="ExternalInput")
    b0_d = nc.dram_tensor("B0COL", [128, 1], F32, kind="ExternalInput")
    b1_d = nc.dram_tensor("B1COL", [128, 1], F32, kind="ExternalInput")
    b2_d = nc.dram_tensor("B2ROW", [1, PPP * 3], F32, kind="ExternalInput")
    out_d = nc.dram_tensor("out", [npc, OUT_DIM], F32, kind="ExternalOutput")

    # DRAM views — partition-contiguous pixel map.
    x_v = x_d[:].rearrange("(p b n) f -> b p (n f)", p=128, b=n_batch)
    m_v = m_d[:].rearrange("(p n) -> p n", p=128)
    o_v = out_d[:].rearrange("(p b n) c -> b p (n c)", p=128, b=n_batch)

    A = mybir.AluOpType

    with tile.TileContext(nc) as tc:
        with (
            tc.tile_pool(name="consts", bufs=1) as cpool,
            tc.tile_pool(name="xin", bufs=3) as xpool,
            tc.tile_pool(name="tsb", bufs=3) as tpool,
            tc.tile_pool(name="hsb", bufs=2) as hpool,
            tc.tile_pool(name="h2sb", bufs=2) as h2pool,
            tc.tile_pool(name="ysb", bufs=2) as ypool,
            tc.tile_pool(name="co", bufs=2) as copool,
            tc.tile_pool(name="uv", bufs=2) as uvpool,
            tc.tile_pool(name="osb", bufs=2) as opool,
            tc.tile_pool(name="mmps", bufs=4, space="PSUM") as mmps_pool,
        ):
            # ---- constants + whole-core mask (u8 -> bf16 cast DMA) ----
            w0t = cpool.tile([128, 128], BF16, tag="w0t")
            w1bd = cpool.tile([128, 128], BF16, tag="w1bd")
            g2 = cpool.tile([128, 24], BF16, tag="g2")
            mask_sb = cpool.tile([128, pppc], BF16, tag="mask")
            ident = cpool.tile([128, 128], BF16, tag="ident")
            nc.sync.dma_start(w0t[:], w0t_d[:])
            nc.sync.dma_start(w1bd[:], w1bd_d[:])
            nc.sync.dma_start(g2[:], g2_d[:])
            nc.sync.dma_start(ident[:], id_d[:])
            if b01_nonzero:
                b0c = cpool.tile([128, 1], F32, tag="b0c")
                b1c = cpool.tile([128, 1], F32, tag="b1c")
                nc.sync.dma_start(b0c[:], b0_d[:])
                nc.sync.dma_start(b1c[:], b1_d[:])
            if b2_nonzero:
                b2r = cpool.tile([1, PPP * 3], F32, tag="b2r")
                nc.sync.dma_start(b2r[:], b2_d[:])

            def relu_evac(dst, src, bias_tile, on_act):
                if on_act == "split":
                    n = src.shape[-1] // 2
                    relu_evac(dst[:, :n], src[:, :n], bias_tile, True)
                    relu_evac(dst[:, n:], src[:, n:], bias_tile, False)
                    return
                if on_act:
                    bias = bias_tile[:] if bias_tile is not None else 0.0
                    nc.scalar.activation(
                        dst, src, mybir.ActivationFunctionType.Relu,
                        bias=bias)
                else:
                    s1 = bias_tile[:] if bias_tile is not None else 0.0
                    nc.vector.tensor_scalar(
                        out=dst, in0=src, scalar1=s1, scalar2=0.0,
                        op0=A.add, op1=A.max)

            # ---- batch schedule: uniform within-partition pixel ranges
            # (ramped batch sizes were tried and hurt the steady state) ----
            sizes = [PPP] * (pppc // PPP)
            assert sum(sizes) == pppc
            spans = []
            off = 0
            for sz in sizes:
                spans.append((off, sz))
                off += sz
            batches = [sp for _ in range(repeat) for sp in spans]

            x_pn = x_d[:].rearrange("(p n) f -> p (n f)", p=128)
            o_pn = out_d[:].rearrange("(p n) c -> p (n c)", p=128)

            x_tiles = {}

            def load_x(i):
                # SWDGE cast f32->bf16, pixel-major
                off, ppb = batches[i]
                x_bf = xpool.tile([128, PPP * IN_DIM], BF16, tag="x")
                nc.gpsimd.dma_start(
                    x_bf[:, :ppb * IN_DIM],
                    x_pn[:, off * IN_DIM:(off + ppb) * IN_DIM])
                x_tiles[i] = x_bf

            def emit_tail(prev, nsplit=1):
                assert prev[0][1] % nsplit == 0
                """v = bco*y (DVE), o = u+v (Pool), store (SP) for a
                FINISHED batch — software-pipelined into the next batch's
                schedule so the sigmoid->v->o chain is off the critical
                inter-batch cycle.  nsplit>1 pipelines the v->o->store
                chain in column chunks (used for the final drain)."""
                (off, ppb), y_p, u_p, bco_p = prev
                v_sb = uvpool.tile([128, PPP * 3], F32, tag="v")
                o_sb = opool.tile([128, PPP * 3], F32, tag="o")
                pc = ppb // nsplit
                for k in range(nsplit):
                    p0, p1 = k * pc, (k + 1) * pc
                    nc.vector.tensor_tensor(
                        out=v_sb[:, p0 * 3:p1 * 3]
                            .rearrange("p (n c) -> p n c", c=3),
                        in0=y_p[:, p0 * 3:p1 * 3]
                            .rearrange("p (n c) -> p n c", c=3),
                        in1=bco_p[:, p0:p1].unsqueeze(2).broadcast_to(
                            [128, pc, 3]),
                        op=A.mult)
                    nc.gpsimd.tensor_tensor(
                        out=o_sb[:, p0 * 3:p1 * 3], in0=u_p[:, p0 * 3:p1 * 3],
                        in1=v_sb[:, p0 * 3:p1 * 3], op=A.add)
                    nc.sync.dma_start(
                        o_pn[:, (off + p0) * 3:(off + p1) * 3],
                        o_sb[:, p0 * 3:p1 * 3])

            load_x(0)
            # mask (u8 -> bf16 cast) split: a small head chunk so batch 0
            # isn't stuck behind the whole-core transfer, then the rest.
            mh = spans[0][1] + spans[1][1] if len(spans) > 1 else pppc
            nc.gpsimd.dma_start(mask_sb[:, :mh], m_v[:, :mh])
            load_x(1)
            if mh < pppc:
                nc.gpsimd.dma_start(mask_sb[:, mh:], m_v[:, mh:])
            prev = None  # ((off, ppb), y_sb, u_sb, bco) of previous batch

            for i, (off, ppb) in enumerate(batches):
                x_bf = x_tiles.pop(i)
                ncols = ppb * IN_DIM           # x/t cols this batch
                # evac tile count per s-half per layer; tile width is
                # always 1024 cols since ppb is a multiple of 128
                assert ppb % 128 == 0
                ntix = ppb // 128

                # ---- feature-major transpose ----
                # t[j, c, g] = x_bf[g, 128c + j]; j = 8*slot + feat
                t_sb = tpool.tile([128, PPP * IN_DIM], BF16, tag="t")
                if i <= 1:
                    # fill-phase batches via PE+DVE through the PSUM ring:
                    # start the moment x lands, well before the XBAR path
                    for half in range(ncols // 2048):
                        t_ps = mmps_pool.tile([128, 2048], BF16, tag="mm")
                        for c in range(16):
                            lo = (half * 16 + c) * 128
                            nc.tensor.transpose(
                                t_ps[:, c * 128:(c + 1) * 128],
                                x_bf[:, lo:lo + 128], ident[:])
                        nc.vector.tensor_copy(
                            t_sb[:, half * 2048:(half + 1) * 2048], t_ps[:])
                else:
                    # XBAR DMA transpose (SP queue)
                    nc.sync.dma_start_transpose(
                        t_sb[:, :ncols].rearrange("j (c g) -> j c g", g=128),
                        x_bf[:, :ncols].rearrange("p (c j) -> p c j", j=128),
                    )

                # ---- previous batch's blend tail + store ----
                if prev is not None:
                    emit_tail(prev)

                # ---- prefetch next batch's x first in Pool's stream, so
                # its SWDGE desc-gen isn't queued behind this batch's TTs ----
                if i + 1 < len(batches) and i + 1 not in x_tiles:
                    load_x(i + 1)

                # ---- blend coefficients early, on Pool ----
                x3 = x_bf[:, :ncols].rearrange("p (n f) -> p n f", f=IN_DIM)
                rgb = x3[:, :, 0:3]
                res = x3[:, :, 3]
                mk = mask_sb[:, off:off + ppb]
                bco = copool.tile([128, PPP], BF16, tag="bc")
                aco = copool.tile([128, PPP], BF16, tag="ac")
                u_sb = uvpool.tile([128, PPP * 3], F32, tag="u")
                nc.gpsimd.tensor_tensor(out=bco[:, :ppb], in0=res, in1=mk,
                                        op=A.mult)
                nc.gpsimd.tensor_tensor(out=aco[:, :ppb], in0=mk,
                                        in1=bco[:, :ppb], op=A.subtract)
                nc.gpsimd.tensor_tensor(
                    out=u_sb[:, :ppb * 3].rearrange("p (n c) -> p n c", c=3),
                    in0=rgb,
                    in1=aco[:, :ppb].unsqueeze(2).broadcast_to([128, ppb, 3]),
                    op=A.mult)

                b0t = b0c if b01_nonzero else None
                b1t = b1c if b01_nonzero else None

                # ---- L0/L1 with interleaved s-halves ----
                h_sb = {}
                h2_sb = {}
                for s in range(2):
                    h_cur = hpool.tile([128, 4096], BF16, tag=f"hs{s}")
                    h_sb[s] = h_cur
                    h2_cur = h2pool.tile([128, 4096], BF16, tag=f"h2s{s}")
                    h2_sb[s] = h2_cur

                tw = 1024                      # evac tile width (cols)
                for tix in range(ntix):
                    for s in range(2):
                        h_ps = mmps_pool.tile([128, 1024], F32, tag="mm")
                        for q in range(2):
                            lo = tix * tw + q * (tw // 2)
                            nc.tensor.matmul(
                                h_ps[:, q * (tw // 2):(q + 1) * (tw // 2)],
                                lhsT=w0t[s * 64:(s + 1) * 64, :],
                                rhs=t_sb[s * 64:(s + 1) * 64, lo:lo + tw // 2],
                            )
                        relu_evac(h_sb[s][:, tix * tw:(tix + 1) * tw],
                                  h_ps[:, :tw], b0t,
                                  EVAC_ON_ACT[(tix * 2 + s) % 8])

                y_sb = ypool.tile([128, PPP * 3], BF16, tag="y")
                cdone = 0  # chunks L2-emitted so far
                for tix in range(ntix):
                    for s in range(2):
                        h2_ps = mmps_pool.tile([128, 1024], F32, tag="mm")
                        for q in range(2):
                            lo = tix * tw + q * (tw // 2)
                            nc.tensor.matmul(
                                h2_ps[:, q * (tw // 2):(q + 1) * (tw // 2)],
                                lhsT=w1bd[:],
                                rhs=h_sb[s][:, lo:lo + tw // 2],
                            )
                        relu_evac(h2_sb[s][:, tix * tw:(tix + 1) * tw],
                                  h2_ps[:, :tw], b1t,
                                  EVAC_ON_ACT[8 + (tix * 2 + s) % 8])

                    # ---- L2 + sigmoid for the chunk range now complete
                    # (z tile [128,768] = up to 16 chunks) ----
                    if tix % 2 == 1 or tix == ntix - 1:
                        cend = (tix + 1) * tw // 128
                        nch = cend - cdone
                        z_ps = mmps_pool.tile([128, 1024], F32, tag="mm")
                        for s in range(2):
                            for cl in range(nch):
                                cc = cdone + cl
                                col = cl * 48 + s * 24
                                nc.tensor.matmul(
                                    z_ps[:, col:col + 24],
                                    lhsT=h2_sb[s][:, cc * 128:(cc + 1) * 128],
                                    rhs=g2[:],
                                )
                        if b2_nonzero:
                            nc.vector.tensor_tensor(
                                out=z_ps[:, :nch * 48],
                                in0=z_ps[:, :nch * 48],
                                in1=b2r[:, cdone * 48:cend * 48]
                                    .partition_broadcast(128),
                                op=A.add)
                        nc.scalar.activation(
                            y_sb[:, cdone * 48:cend * 48], z_ps[:, :nch * 48],
                            mybir.ActivationFunctionType.Sigmoid,
                        )
                        cdone = cend

                prev = ((off, ppb), y_sb, u_sb, bco)

            # final batch's tail, split to pipeline the drain
            emit_tail(prev, nsplit=4)

    nc.finalize()
    return nc


_PROGRAM_CACHE = {}


def _get_program(npc, b01_nonzero, b2_nonzero, repeat=1):
    key = (npc, b01_nonzero, b2_nonzero, repeat)
    if key not in _PROGRAM_CACHE:
        _PROGRAM_CACHE[key] = build_program(npc, b01_nonzero, b2_nonzero,
                                            repeat=repeat)
    return _PROGRAM_CACHE[key]


def _shard_inputs(x, mask):
    x_flat = np.ascontiguousarray(np.asarray(x, np.float32).reshape(NPX, IN_DIM))
    m_flat = np.ascontiguousarray(np.asarray(mask).astype(np.uint8).reshape(NPX))
    return x_flat, m_flat


def kernel(x, mask, w0, b0, w1, b1, w2, b2):
    consts = _prep_weights(
        np.asarray(w0, np.float32), np.asarray(b0, np.float32),
        np.asarray(w1, np.float32), np.asarray(b1, np.float32),
        np.asarray(w2, np.float32), np.asarray(b2, np.float32))
    x_flat, m_flat = _shard_inputs(x, mask)

    nc = _get_program(NPC, consts["b01_nonzero"], consts["b2_nonzero"])
    const_map = {k: np.asarray(v) for k, v in consts.items()
                 if k not in ("b2_nonzero", "b01_nonzero")}
    in_maps = []
    for k in range(N_CORES):
        lo, hi = k * NPC, (k + 1) * NPC
        in_maps.append({
            "x": x_flat[lo:hi],
            "mask": m_flat[lo:hi],
            **const_map,
        })
    res = run_bass_kernel_spmd(nc, in_maps, core_ids=list(range(N_CORES)))
    out = np.concatenate([res.results[k]["out"] for k in range(N_CORES)], axis=0)
    return out.reshape(B, H, W, OUT_DIM)


if __name__ == "__main__":
    rng = np.random.default_rng(0)
    x = rng.random((B, H, W, IN_DIM), np.float32)
    mask = rng.integers(0, 2, (B, H, W)).astype(bool)
    w0 = rng.standard_normal((IN_DIM, HID)).astype(np.float32) * 0.5
    b0 = np.zeros(HID, np.float32)
    w1 = rng.standard_normal((HID, HID)).astype(np.float32) * 0.3
    b1 = np.zeros(HID, np.float32)
    w2 = rng.standard_normal((HID, OUT_DIM)).astype(np.float32) * 0.3
    b2 = np.zeros(OUT_DIM, np.float32)
    out = kernel(x=x, mask=mask, w0=w0, b0=b0, w1=w1, b1=b1, w2=w2, b2=b2)
    print("out", out.shape, out.dtype, out[0, 0, :2])
